# revision 11
# baseline (speedup 1.0000x reference)
"""Chamfer-style loss kernel for Trainium2 (8 NeuronCores, SPMD) — v6.

Problem: y_pred [8192,2], y_true [8192,2] (fp32).
  d[n,m] = ||p_n - t_m||;  loss = (sum_n min_m d + sum_m min_n d) / 8192

Strategy per core k (shard y_pred rows, 1024 per core):
  - S[n,m] = |p|^2 + |t|^2 - 2 p.t via an augmented K=18 bf16 matmul:
    each fp32 value is split into three bf16 terms on the host; kept
    partial products bound the error at ~2^-23 |p||t|, far below the
    bf16 rounding of S.  bf16 runs at 1 cycle/row (fp32 is 4) with no
    quadrant packing, so no 4x lhs/rhs partition replication.
  - ACT copies PSUM->SBUF bf16; per pair the DVE does only
    X = min(chunk a, chunk b); rowacc = min(rowacc, X); and the
    3-level block tree to t3.  Batched DMA transposes are issued
    mid-loop but nothing in the loop consumes them: collectives block
    the DMA rings from trigger to completion (measured 20..70us,
    highly variable), so every transpose-dependent fold runs in the
    tail.  t3/tbp tiles are persistent to keep pool rotation from
    serializing against delayed transposes.
  - Collective warm-up: a dummy [128,1] AllGather at pair 1 absorbs
    the 11-30us first-collective trigger delay and core skew; nothing
    depends on it.  The single real AllGather at the end then starts
    with ~1.2us delay and runs ~7-10us: payload [128,72] = 64 folded
    col-min columns + 8 per-block row mins.
  - Post-AG: one gather DMA, clamp, 8-way min tree, sqrt with
    free-axis accumulation, PSUM-accumulated ones-matmul partition
    sums, scale by 1/8192.  Consumers are dep-pinned behind the AG
    payload / last tail DVE op so the static scheduler cannot park
    them at a queue head mid-loop.
"""

import sys

if "/opt/trn_rl_repo" not in sys.path:
    sys.path.insert(0, "/opt/trn_rl_repo")

import ml_dtypes
import numpy as np

import concourse.bass as bass
import concourse.bacc as bacc
import concourse.tile as tile
from concourse import mybir
from concourse.bass_utils import run_bass_kernel_spmd
from concourse.tile_rust import add_dep_helper

F32 = mybir.dt.float32
BF16 = mybir.dt.bfloat16
MIN = mybir.AluOpType.min
ADD = mybir.AluOpType.add
X = mybir.AxisListType.X

N_CORES = 8
N = 8192
M = 8192
N_LOC = N // N_CORES     # 1024 rows per core
N_BLK = N_LOC // 128     # 8 row blocks
N_PAIR = 8               # pairs of 512-column chunks
CHUNK = 512
K_AUG = 18               # augmented contraction depth

TRACE = False
LAST_RESULTS = None

_CACHE = {}


def _build_program():
    nc = bacc.Bacc(
        "TRN2",
        target_bir_lowering=False,
        debug=False,
        num_devices=N_CORES,
    )

    lhs_d = nc.dram_tensor("lhs", [K_AUG, N_LOC], BF16, kind="ExternalInput")
    rhs_d = nc.dram_tensor("rhs", [K_AUG, M], BF16, kind="ExternalInput")
    out_d = nc.dram_tensor("out", [1, 1], F32, kind="ExternalOutput")

    with tile.TileContext(nc) as tc:
        with (
            tc.tile_pool(name="const", bufs=1) as const_pool,
            tc.tile_pool(name="acc", bufs=1) as acc_pool,
            tc.tile_pool(name="chunk", bufs=3) as chunk_pool,
            tc.tile_pool(name="tree", bufs=2) as tree_pool,
            tc.tile_pool(name="fin", bufs=1) as fin_pool,
            tc.tile_pool(name="mm", bufs=2, space="PSUM") as mm_pool,
            tc.tile_pool(name="dram", bufs=1, space="DRAM") as dram_pool,
        ):
            # ---- constants / inputs ----
            lhs_sb = const_pool.tile([K_AUG, N_LOC], BF16)
            rhs_sb = const_pool.tile([K_AUG, M], BF16)
            ones_sb = const_pool.tile([128, 1], F32)

            nc.scalar.dma_start(lhs_sb[:, :], lhs_d.ap())
            in_dmas = []
            for p in range(N_PAIR):
                lo, hi = p * 1024, (p + 1) * 1024
                eng = nc.sync if p % 2 == 0 else nc.scalar
                in_dmas.append(
                    eng.dma_start(rhs_sb[:, lo:hi], rhs_d.ap()[:, lo:hi])
                )
            nc.vector.memset(ones_sb[:, :], 1.0)

            # ---- persistent accumulators ----
            rowacc_a = acc_pool.tile([128, N_BLK * CHUNK], BF16)
            rowacc_b = acc_pool.tile([128, N_BLK * CHUNK], BF16)
            rowaccs = [rowacc_a, rowacc_b]
            # colc[a, 8p+j] = local col-min of column 1024p + 128j + a,
            # cols 64:72 = per-block row mins
            colc = acc_pool.tile([128, 72], BF16)

            sync_in = dram_pool.tile([128, 1], F32)
            sync_out = dram_pool.tile([1024, 1], F32, addr_space="Shared")
            ag_in = dram_pool.tile([128, 72], BF16)
            ag_out = dram_pool.tile([1024, 72], BF16, addr_space="Shared")

            t3s = [None] * N_PAIR
            tbps = [None] * N_PAIR

            # ---- main loop over pairs of chunks ----
            for p in range(N_PAIR):
                pair_sb = chunk_pool.tile(
                    [128, 2 * N_BLK * CHUNK], BF16, name="pair_sb", tag="chunk"
                )
                for h in range(4):          # (chunk, blockgroup)
                    c = 2 * p + h // 2
                    g = h % 2
                    mm_ps = mm_pool.tile(
                        [128, 4 * CHUNK], F32, name="mm_ps", tag="mm"
                    )
                    for r in range(4):
                        i = 4 * g + r
                        nc.tensor.matmul(
                            mm_ps[:, r * CHUNK:(r + 1) * CHUNK],
                            lhs_sb[:, i * 128:(i + 1) * 128],
                            rhs_sb[:, c * CHUNK:(c + 1) * CHUNK],
                            start=True,
                            stop=True,
                        )
                    nc.scalar.copy(
                        pair_sb[:, h * 2048:(h + 1) * 2048], mm_ps[:, :]
                    )

                # row path
                if p == 0:
                    nc.vector.tensor_tensor(
                        rowaccs[1][:, :],
                        pair_sb[:, 0:4096],
                        pair_sb[:, 4096:8192],
                        MIN,
                    )
                else:
                    xt = tree_pool.tile([128, 4096], BF16, name="xt", tag="xt")
                    nc.vector.tensor_tensor(
                        xt[:, :], pair_sb[:, 0:4096], pair_sb[:, 4096:8192], MIN
                    )
                    nc.vector.tensor_tensor(
                        rowaccs[(p + 1) % 2][:, :],
                        rowaccs[p % 2][:, :],
                        xt[:, :],
                        MIN,
                    )

                # col path tree
                pv = pair_sb.rearrange("q (c g f) -> q c g f", c=2, g=2)
                t1 = tree_pool.tile([128, 4096], BF16, name="t1", tag="t1")
                nc.vector.tensor_tensor(
                    t1.rearrange("q (c f) -> q c f", c=2),
                    pv[:, :, 0, :],
                    pv[:, :, 1, :],
                    MIN,
                )
                t1v = t1.rearrange("q (c b f) -> q c b f", c=2, b=2)
                t2 = tree_pool.tile([128, 2048], BF16, name="t2", tag="t2")
                nc.vector.tensor_tensor(
                    t2.rearrange("q (c f) -> q c f", c=2),
                    t1v[:, :, 0, :],
                    t1v[:, :, 1, :],
                    MIN,
                )
                t2v = t2.rearrange("q (c b f) -> q c b f", c=2, b=2)
                t3 = fin_pool.tile([128, 1024], BF16, name=f"t3_{p}")
                t3s[p] = t3
                nc.vector.tensor_tensor(
                    t3.rearrange("q (c f) -> q c f", c=2),
                    t2v[:, :, 0, :],
                    t2v[:, :, 1, :],
                    MIN,
                )
                # batched transpose on the DMA ring; consumed only in the
                # tail (a collective may delay it arbitrarily)
                tbp = fin_pool.tile([128, 1024], BF16, name=f"tbp_{p}")
                tbps[p] = tbp
                teng = nc.sync if p % 2 == 0 else nc.scalar
                teng.dma_start_transpose(
                    tbp.rearrange("a (j b) -> a j b", j=8), t3[:, :]
                )

                if p == 2:
                    # dummy collective: absorbs first-collective trigger
                    # delay + core skew; nothing depends on it.  It must
                    # not fire while input DMAs are in flight (collectives
                    # freeze every DMA ring from trigger to completion).
                    i_syncpay = nc.scalar.dma_start(
                        sync_in[:, :], ones_sb[:, 0:1]
                    )
                    for dma in in_dmas:
                        add_dep_helper(i_syncpay.ins, dma.ins, sync=True,
                                       reason="warmup AG after input DMAs")
                    nc.gpsimd.collective_compute(
                        "AllGather",
                        mybir.AluOpType.bypass,
                        replica_groups=[list(range(N_CORES))],
                        ins=[sync_in[:, :].opt()],
                        outs=[sync_out[:, :].opt()],
                    )

            # ---- loop tail ----
            # row-min tree over rowacc
            racc = rowaccs[(N_PAIR) % 2]
            rv = racc.rearrange("q (b f) -> q b f", b=N_BLK)
            r1 = fin_pool.tile([128, N_BLK * 256], BF16)
            nc.vector.tensor_tensor(
                r1.rearrange("q (b f) -> q b f", b=N_BLK),
                rv[:, :, 0:256],
                rv[:, :, 256:512],
                MIN,
            )
            r1v = r1.rearrange("q (b f) -> q b f", b=N_BLK)
            r2 = fin_pool.tile([128, N_BLK * 128], BF16)
            nc.vector.tensor_tensor(
                r2.rearrange("q (b f) -> q b f", b=N_BLK),
                r1v[:, :, 0:128],
                r1v[:, :, 128:256],
                MIN,
            )
            nc.vector.tensor_reduce(
                colc[:, 64:72],
                r2.rearrange("q (b f) -> q b f", b=N_BLK),
                axis=X,
                op=MIN,
            )
            # all 8 folds (transposes have long drained by loop end)
            i_fold_last = None
            for p in range(N_PAIR):
                i_fold_last = nc.vector.tensor_reduce(
                    colc[:, 8 * p:8 * p + 8],
                    tbps[p].rearrange("a (j b) -> a j b", j=8),
                    axis=X,
                    op=MIN,
                )

            i_agpay = nc.scalar.dma_start(ag_in[:, :], colc[:, 0:72])
            nc.gpsimd.collective_compute(
                "AllGather",
                mybir.AluOpType.bypass,
                replica_groups=[list(range(N_CORES))],
                ins=[ag_in[:, :].opt()],
                outs=[ag_out[:, :].opt()],
            )

            # ---- post-AG finalization (identical on every core) ----
            call = fin_pool.tile([128, 576], BF16)
            i_l = nc.sync.dma_start(
                call.rearrange("q (j c) -> q j c", j=N_CORES),
                ag_out.rearrange("(j q) c -> q j c", j=N_CORES),
            )
            add_dep_helper(i_l.ins, i_agpay.ins, sync=False,
                           reason="AG consumer after loop tail")

            # clamp everything once (col mins + row mins)
            i_cl = nc.vector.tensor_scalar_max(call[:, :], call[:, :], 0.0)
            add_dep_helper(i_cl.ins, i_fold_last.ins, sync=False,
                           reason="post-AG clamp after last fold")

            # 8-way min tree over cores for the 64 col-min columns
            v = call.rearrange("q (j c) -> q j c", j=N_CORES)
            m1 = fin_pool.tile([128, 256], BF16)
            nc.vector.tensor_tensor(
                m1.rearrange("q (j c) -> q j c", j=4),
                v[:, 0:4, 0:64],
                v[:, 4:8, 0:64],
                MIN,
            )
            m1v = m1.rearrange("q (j c) -> q j c", j=4)
            m2 = fin_pool.tile([128, 128], BF16)
            nc.vector.tensor_tensor(
                m2.rearrange("q (j c) -> q j c", j=2),
                m1v[:, 0:2, :],
                m1v[:, 2:4, :],
                MIN,
            )
            m2v = m2.rearrange("q (j c) -> q j c", j=2)
            cmin = fin_pool.tile([128, 64], BF16)
            nc.vector.tensor_tensor(
                cmin.rearrange("q (j c) -> q j c", j=1),
                m2v[:, 0:1, :],
                m2v[:, 1:2, :],
                MIN,
            )

            cd = fin_pool.tile([128, 64], F32)
            colpart = fin_pool.tile([128, 1], F32)
            nc.scalar.activation(
                cd[:, :], cmin[:, :],
                mybir.ActivationFunctionType.Sqrt,
                accum_out=colpart[:, :],
            )
            rowd = fin_pool.tile([128, 64], F32)
            rowpart = fin_pool.tile([128, 1], F32)
            nc.scalar.activation(
                rowd[:, :], v[:, :, 64:72],
                mybir.ActivationFunctionType.Sqrt,
                accum_out=rowpart[:, :],
            )

            ps_fin = mm_pool.tile([128, 4 * CHUNK], F32, name="ps_fin", tag="mm")
            nc.tensor.matmul(
                ps_fin[0:1, 0:1], ones_sb[:, :], colpart[:, :],
                start=True, stop=False,
            )
            nc.tensor.matmul(
                ps_fin[0:1, 0:1], ones_sb[:, :], rowpart[:, :],
                start=False, stop=True,
            )
            sc = fin_pool.tile([1, 1], F32)
            nc.scalar.copy(sc[:, :], ps_fin[0:1, 0:1])
            out_sb = fin_pool.tile([1, 1], F32)
            nc.scalar.mul(out_sb[:, :], sc[:, :], 1.0 / M)
            nc.sync.dma_start(out_d.ap(), out_sb[:, :])

    nc.compile()
    return nc


def _split3(x):
    """Split fp64 array into three bf16 terms h+m+l with ~2^-24 residual."""
    h = x.astype(ml_dtypes.bfloat16)
    r = x - h.astype(np.float64)
    m = r.astype(ml_dtypes.bfloat16)
    l = (r - m.astype(np.float64)).astype(ml_dtypes.bfloat16)
    return h, m, l


def _prep_inputs(y_pred, y_true):
    p = np.asarray(y_pred, dtype=np.float64).reshape(-1, 2)
    t = np.asarray(y_true, dtype=np.float64).reshape(-1, 2)
    assert p.shape == (N, 2) and t.shape == (M, 2)

    thx, tmx, tlx = _split3(t[:, 0])
    thy, tmy, tly = _split3(t[:, 1])
    nth, ntm, ntl = _split3(t[:, 0] ** 2 + t[:, 1] ** 2)
    one_t = np.ones(M, dtype=ml_dtypes.bfloat16)

    rhs = np.empty((K_AUG, M), dtype=ml_dtypes.bfloat16)
    rhs[0] = thx
    rhs[1] = tmx
    rhs[2] = thx
    rhs[3] = tmx
    rhs[4] = tlx
    rhs[5] = thx
    rhs[6] = thy
    rhs[7] = tmy
    rhs[8] = thy
    rhs[9] = tmy
    rhs[10] = tly
    rhs[11] = thy
    rhs[12] = one_t
    rhs[13] = one_t
    rhs[14] = one_t
    rhs[15] = nth
    rhs[16] = ntm
    rhs[17] = ntl

    in_maps = []
    for k in range(N_CORES):
        pk = p[k * N_LOC:(k + 1) * N_LOC]
        phx, pmx, plx = _split3(-2.0 * pk[:, 0])
        phy, pmy, ply = _split3(-2.0 * pk[:, 1])
        nph, npm, npl = _split3(pk[:, 0] ** 2 + pk[:, 1] ** 2)
        one_p = np.ones(N_LOC, dtype=ml_dtypes.bfloat16)

        lhs = np.empty((K_AUG, N_LOC), dtype=ml_dtypes.bfloat16)
        lhs[0] = phx
        lhs[1] = phx
        lhs[2] = pmx
        lhs[3] = pmx
        lhs[4] = phx
        lhs[5] = plx
        lhs[6] = phy
        lhs[7] = phy
        lhs[8] = pmy
        lhs[9] = pmy
        lhs[10] = phy
        lhs[11] = ply
        lhs[12] = nph
        lhs[13] = npm
        lhs[14] = npl
        lhs[15] = one_p
        lhs[16] = one_p
        lhs[17] = one_p
        in_maps.append({"lhs": lhs, "rhs": rhs})
    return in_maps


def kernel(y_pred, y_true):
    global LAST_RESULTS
    if "nc" not in _CACHE:
        _CACHE["nc"] = _build_program()
    nc = _CACHE["nc"]
    in_maps = _prep_inputs(y_pred, y_true)
    res = run_bass_kernel_spmd(
        nc,
        in_maps,
        core_ids=list(range(N_CORES)),
        trace=TRACE,
    )
    LAST_RESULTS = res
    return np.asarray(res.results[0]["out"], dtype=np.float32).reshape(())[()]
